# revision 6
# baseline (speedup 1.0000x reference)
"""Multi-head latent attention (DeepSeek-V2 style MLA) on 8 Trainium2 NeuronCores.

Sharding: data-parallel over batch (2 groups of 4 cores). Within each group,
tensor-parallel over heads (4 heads/core); the low-rank a-projections are
token-sharded (512 tokens/core) and all-gathered; the o-projection is
column-parallel after an all-gather of per-head attention outputs.

All matmuls run in float32r (hardware tf32-like, ~11-bit mantissa rounding,
full PE throughput); softmax/norm arithmetic stays fp32.
"""

import numpy as np

import concourse.mybir as mybir
import concourse.tile as tile
from concourse import bacc
from concourse.bass_utils import run_bass_kernel_spmd

# problem dims (hardcoded per contract)
S = 2048
B = 2
HID = 2048
NH = 16
DQN = 128
DR = 64
DV = 128
QLR = 1536
KVLR = 512
DQ = DQN + DR  # 192
EPS = 1e-6
BASE = 10000.0

P = 128
N_CORES = 8
LANES = 4                  # tensor-parallel width within a batch group
GROUPS = [[0, 1, 2, 3], [4, 5, 6, 7]]
TOKS = S // LANES          # 512 tokens/core in stage A
HPC = NH // LANES          # 4 heads per core
QT = 512                   # attention query tile
NQT = S // QT              # 4
NKB = S // P               # 16 key blocks
KH = HID // P              # 16
KQ = QLR // P              # 12
KC = KVLR // P             # 4
AG1_ROWS = QLR + KVLR + DR  # 2112

F32 = mybir.dt.float32
F32R = mybir.dt.float32r
AF = mybir.ActivationFunctionType
OP = mybir.AluOpType

# True: DVE may read psum at base-partition 64 against an sbuf operand at
# base 0 (verified on HW).  False: bounce the psum upper half through a DMA.
MIXED_BASE_PSUM = True

_NC_CACHE = None


def _rope_combine(nc, pool, out_sb, ps, cs, sn, width, tag):
    """out[0:64] = ps[0:64]*cs + ps[64:128]*sn  (all [64, width])."""
    t1 = pool.tile([DR, width], F32, tag=f"{tag}_t1")
    nc.vector.tensor_mul(t1[:], ps[0:DR, :], cs)
    t2 = pool.tile([DR, width], F32, tag=f"{tag}_t2")
    if MIXED_BASE_PSUM:
        nc.vector.tensor_mul(t2[:], ps[DR:2 * DR, :], sn)
    else:
        rt = pool.tile([DR, width], F32, tag=f"{tag}_rt")
        nc.sync.dma_start(rt[:], ps[DR:2 * DR, :])
        nc.vector.tensor_mul(t2[:], rt[:], sn)
    nc.vector.tensor_add(out_sb[:], t1[:], t2[:])


def _build():
    nc = bacc.Bacc("TRN2", target_bir_lowering=False, debug=False,
                   num_devices=N_CORES)

    # per-core external inputs (host pre-sharded / pre-tiled)
    hid_t = nc.dram_tensor("hid_t", [HID, TOKS], F32R, kind="ExternalInput")
    wqa_t = nc.dram_tensor("wqa_t", [KQ * P, KH * P], F32R, kind="ExternalInput")
    wkva_t = nc.dram_tensor("wkva_t", [KC * P, KH * P], F32R, kind="ExternalInput")
    wkva_pe = nc.dram_tensor("wkva_pe", [P, KH * P], F32R, kind="ExternalInput")
    wqb_t = nc.dram_tensor("wqb_t", [2 * HPC * P, KQ * P], F32R, kind="ExternalInput")
    wkvb_k = nc.dram_tensor("wkvb_k", [HPC * P, KC * P], F32R, kind="ExternalInput")
    wv = nc.dram_tensor("wv", [KC * P, HPC * DV], F32R, kind="ExternalInput")
    wo_t = nc.dram_tensor("wo_t", [LANES * P, KH * P], F32R, kind="ExternalInput")
    cos_k = nc.dram_tensor("cos_k", [DR, TOKS], F32, kind="ExternalInput")
    sin_k = nc.dram_tensor("sin_k", [DR, TOKS], F32, kind="ExternalInput")
    cos_q = nc.dram_tensor("cos_q", [DR, S], F32, kind="ExternalInput")
    sin_q = nc.dram_tensor("sin_q", [DR, S], F32, kind="ExternalInput")
    maskp = nc.dram_tensor("maskp", [LANES * P, QT], F32, kind="ExternalInput")
    o_t = nc.dram_tensor("o_t", [LANES * P, S], F32, kind="ExternalOutput")

    with tile.TileContext(nc) as tc:
        with (
            tc.tile_pool(name="dram", bufs=1, space="DRAM") as dpool,
            tc.tile_pool(name="const", bufs=1) as cpool,
        ):
            ag1_in = dpool.tile([AG1_ROWS, TOKS], F32R)
            ag1_out = dpool.tile([LANES * AG1_ROWS, TOKS], F32R)
            q_spill = dpool.tile([HPC * DQ, S], F32R)
            o_bounce = dpool.tile([HPC * DV, S], F32R)
            o_gath = dpool.tile([NH * DV, S], F32R)

            ones_f = cpool.tile([P, 1], F32)
            nc.vector.memset(ones_f[:], 1.0)
            ones128 = cpool.tile([P, 1], F32R)      # lhsT for partition-sum
            nc.vector.tensor_copy(ones128[:], ones_f[:])
            ones_f1 = cpool.tile([1, P], F32)
            nc.vector.memset(ones_f1[:], 1.0)
            ones1 = cpool.tile([1, P], F32R)        # lhsT for partition-bcast
            nc.vector.tensor_copy(ones1[:], ones_f1[:])
            eps_sb = cpool.tile([1, 1], F32)
            nc.vector.memset(eps_sb[:], EPS)

            # ---------------- stage A: a-projections + RMSNorm scales -------
            with (
                tc.tile_pool(name="a_hid", bufs=1) as hidp,
                tc.tile_pool(name="a_w", bufs=3) as awp,
                tc.tile_pool(name="a_raw", bufs=1) as rawp,
                tc.tile_pool(name="a_sq", bufs=2) as sqp,
                tc.tile_pool(name="a_ps", bufs=2, space="PSUM") as apsp,
                tc.tile_pool(name="a_ss", bufs=1, space="PSUM") as assp,
                tc.tile_pool(name="a_bc", bufs=1, space="PSUM") as abcp,
                tc.tile_pool(name="a_misc", bufs=2) as amp,
            ):
                hid_sb = hidp.tile([P, KH * TOKS], F32R)
                for k in range(KH):
                    nc.sync.dma_start(hid_sb[:, k * TOKS:(k + 1) * TOKS],
                                      hid_t[k * P:(k + 1) * P, :])
                csk = hidp.tile([DR, TOKS], F32)
                nc.sync.dma_start(csk[:], cos_k[:, :])
                snk = hidp.tile([DR, TOKS], F32)
                nc.sync.dma_start(snk[:], sin_k[:, :])

                ss_q = assp.tile([1, TOKS], F32, tag="ssq")
                ss_kv = assp.tile([1, TOKS], F32, tag="sskv")
                raws = []
                NBLK = KQ + KC  # 16 normed blocks
                for bi in range(NBLK + 1):
                    w_sb = awp.tile([P, KH * P], F32R, tag="aw")
                    if bi < KQ:
                        src = wqa_t[bi * P:(bi + 1) * P, :]
                    elif bi < NBLK:
                        src = wkva_t[(bi - KQ) * P:(bi - KQ + 1) * P, :]
                    else:
                        src = wkva_pe[:, :]
                    nc.sync.dma_start(w_sb[:], src)
                    ps = apsp.tile([P, TOKS], F32, tag="aps")
                    for k in range(KH):
                        nc.tensor.matmul(
                            ps[:], w_sb[:, k * P:(k + 1) * P],
                            hid_sb[:, k * TOKS:(k + 1) * TOKS],
                            start=(k == 0), stop=(k == KH - 1))
                    if bi < NBLK:
                        r_sb = rawp.tile([P, TOKS], F32, tag=f"raw{bi}")
                        nc.vector.tensor_copy(r_sb[:], ps[:])
                        raws.append(r_sb)
                        sq = sqp.tile([P, TOKS], F32R, tag="sq")
                        nc.scalar.activation(sq[:], ps[:], AF.Square)
                        tgt = ss_q if bi < KQ else ss_kv
                        nc.tensor.matmul(
                            tgt[:], ones128[:], sq[:],
                            start=(bi in (0, KQ)),
                            stop=(bi in (KQ - 1, NBLK - 1)),
                            skip_group_check=True)
                    else:
                        kpe_sb = amp.tile([DR, TOKS], F32R, tag="kpe")
                        _rope_combine(nc, amp, kpe_sb, ps, csk[:], snk[:],
                                      TOKS, "akpe")
                        nc.sync.dma_start(ag1_in[QLR + KVLR:AG1_ROWS, :],
                                          kpe_sb[:])

                # rms scales: 1/sqrt(mean(x^2) + eps), broadcast to 128 parts
                def norm_scale(ss_ps, dim, tag):
                    tmp = amp.tile([1, TOKS], F32, tag=f"nt_{tag}")
                    nc.scalar.activation(tmp[:], ss_ps[:], AF.Sqrt,
                                         bias=eps_sb[:], scale=1.0 / dim)
                    s_sb = amp.tile([1, TOKS], F32R, tag=f"ns_{tag}")
                    with nc.allow_low_precision(
                            reason="f32r rounding of rms scale is intended"):
                        nc.vector.reciprocal(s_sb[:], tmp[:])
                    bc = abcp.tile([P, TOKS], F32, tag=f"nb_{tag}")
                    nc.tensor.matmul(bc[:], ones1[:], s_sb[:],
                                     start=True, stop=True,
                                     skip_group_check=True)
                    return bc

                bq = norm_scale(ss_q, QLR, "q")
                bkv = norm_scale(ss_kv, KVLR, "kv")

                for bi in range(NBLK):
                    bc = bq if bi < KQ else bkv
                    sc_sb = sqp.tile([P, TOKS], F32R, tag="scaled")
                    nc.vector.tensor_tensor(sc_sb[:], raws[bi][:], bc[:],
                                            OP.mult)
                    nc.sync.dma_start(ag1_in[bi * P:(bi + 1) * P, :], sc_sb[:])

            # ---------------- all-gather latents ---------------------------
            nc.gpsimd.collective_compute(
                "AllGather", OP.bypass, replica_groups=GROUPS,
                ins=[ag1_in[:].opt()], outs=[ag1_out[:].opt()])

            # persistent stage-B tiles
            with tc.tile_pool(name="b_keep", bufs=1) as keep:
                knope_sb = [keep.tile([P, S], F32R, tag=f"kn{h}",
                                      name=f"knope{h}")
                            for h in range(HPC)]
                kpe_sb = keep.tile([DR, S], F32R, tag="kpe")
                v_sb = keep.tile([P, NKB * HPC * DV], F32R, tag="v")
                maskp_sb = keep.tile([P, LANES * QT], F32, tag="mask")
                for i in range(LANES):
                    nc.sync.dma_start(maskp_sb[:, i * QT:(i + 1) * QT],
                                      maskp[i * P:(i + 1) * P, :])

                # ---------- B1: q_b projection + rope, spill to DRAM --------
                with (
                    tc.tile_pool(name="b1_w", bufs=1) as b1w,
                    tc.tile_pool(name="b1_r", bufs=13) as b1r,
                    tc.tile_pool(name="b1_ps", bufs=2, space="PSUM") as b1ps,
                    tc.tile_pool(name="b1_t", bufs=3) as b1t,
                ):
                    csq = b1w.tile([DR, S], F32, tag="csq")
                    nc.sync.dma_start(csq[:], cos_q[:, :])
                    snq = b1w.tile([DR, S], F32, tag="snq")
                    nc.sync.dma_start(snq[:], sin_q[:, :])
                    wqb_sb = [b1w.tile([P, KQ * P], F32R, tag=f"wqb{m}",
                                       name=f"wqb_sb{m}")
                              for m in range(2 * HPC)]
                    for m in range(2 * HPC):
                        nc.sync.dma_start(wqb_sb[m][:],
                                          wqb_t[m * P:(m + 1) * P, :])
                    for n in range(NQT):
                        nsl = slice(n * QT, (n + 1) * QT)
                        ql = []
                        for k in range(KQ):
                            t = b1r.tile([P, QT], F32R, tag="ql")
                            r0 = n * AG1_ROWS + k * P
                            nc.sync.dma_start(t[:], ag1_out[r0:r0 + P, :])
                            ql.append(t)
                        for h in range(HPC):
                            ps = b1ps.tile([P, QT], F32, tag="b1ps")
                            for k in range(KQ):
                                nc.tensor.matmul(
                                    ps[:], wqb_sb[2 * h][:, k * P:(k + 1) * P],
                                    ql[k][:], start=(k == 0),
                                    stop=(k == KQ - 1))
                            qn_sb = b1t.tile([P, QT], F32R, tag="qn")
                            nc.vector.tensor_copy(qn_sb[:], ps[:])
                            nc.sync.dma_start(
                                q_spill[h * DQ:h * DQ + P, nsl], qn_sb[:])
                            ps2 = b1ps.tile([P, QT], F32, tag="b1ps")
                            for k in range(KQ):
                                nc.tensor.matmul(
                                    ps2[:],
                                    wqb_sb[2 * h + 1][:, k * P:(k + 1) * P],
                                    ql[k][:], start=(k == 0),
                                    stop=(k == KQ - 1))
                            qp_sb = b1t.tile([DR, QT], F32R, tag="qp")
                            _rope_combine(nc, b1t, qp_sb, ps2,
                                          csq[:, nsl], snq[:, nsl], QT, "qpe")
                            nc.sync.dma_start(
                                q_spill[h * DQ + P:(h + 1) * DQ, nsl],
                                qp_sb[:])

                # ---------- B2+B3: k_nope, v projections --------------------
                with (
                    tc.tile_pool(name="b2_w", bufs=1) as b2w,
                    tc.tile_pool(name="b2_r", bufs=5) as b2r,
                    tc.tile_pool(name="b2_ps", bufs=2, space="PSUM") as b2ps,
                ):
                    wkn_sb = [b2w.tile([P, KC * P], F32R, tag=f"wkn{h}",
                                       name=f"wkn_sb{h}")
                              for h in range(HPC)]
                    for h in range(HPC):
                        nc.sync.dma_start(wkn_sb[h][:],
                                          wkvb_k[h * P:(h + 1) * P, :])
                    wv_sb = b2w.tile([P, KC * HPC * DV], F32R, tag="wv")
                    for k in range(KC):
                        nc.sync.dma_start(
                            wv_sb[:, k * HPC * DV:(k + 1) * HPC * DV],
                            wv[k * P:(k + 1) * P, :])
                    for n in range(LANES):
                        nsl = slice(n * QT, (n + 1) * QT)
                        ckv = []
                        for k in range(KC):
                            t = b2r.tile([P, QT], F32R, tag="ckv")
                            r0 = n * AG1_ROWS + QLR + k * P
                            nc.sync.dma_start(t[:], ag1_out[r0:r0 + P, :])
                            ckv.append(t)
                        r0 = n * AG1_ROWS + QLR + KVLR
                        nc.sync.dma_start(kpe_sb[:, nsl],
                                          ag1_out[r0:r0 + DR, :])
                        for h in range(HPC):
                            ps = b2ps.tile([P, QT], F32, tag="b2ps")
                            for k in range(KC):
                                nc.tensor.matmul(
                                    ps[:], wkn_sb[h][:, k * P:(k + 1) * P],
                                    ckv[k][:], start=(k == 0),
                                    stop=(k == KC - 1))
                            nc.vector.tensor_copy(knope_sb[h][:, nsl], ps[:])
                        for j in range(LANES):
                            ps = b2ps.tile([P, HPC * DV], F32, tag="b2ps")
                            for k in range(KC):
                                nc.tensor.matmul(
                                    ps[:], ckv[k][:, j * P:(j + 1) * P],
                                    wv_sb[:, k * HPC * DV:(k + 1) * HPC * DV],
                                    start=(k == 0), stop=(k == KC - 1))
                            kb = n * LANES + j
                            nc.vector.tensor_copy(
                                v_sb[:, kb * HPC * DV:(kb + 1) * HPC * DV],
                                ps[:])

                # ---------- B5: causal attention ----------------------------
                with (
                    tc.tile_pool(name="b5_q", bufs=3) as b5q,
                    tc.tile_pool(name="b5_e", bufs=4) as b5e,
                    tc.tile_pool(name="b5_sc", bufs=2, space="PSUM") as scp,
                    tc.tile_pool(name="b5_av", bufs=2, space="PSUM") as avp,
                    tc.tile_pool(name="b5_dn", bufs=2, space="PSUM") as dnp,
                    tc.tile_pool(name="b5_bc", bufs=1, space="PSUM") as bcp,
                    tc.tile_pool(name="b5_t", bufs=2) as b5t,
                ):
                    for h in range(HPC):
                        for qt in range(NQT):
                            qsl = slice(qt * QT, (qt + 1) * QT)
                            qn = b5q.tile([P, QT], F32R, tag="qn")
                            nc.sync.dma_start(qn[:],
                                              q_spill[h * DQ:h * DQ + P, qsl])
                            qp = b5q.tile([DR, QT], F32R, tag="qp")
                            nc.sync.dma_start(
                                qp[:], q_spill[h * DQ + P:(h + 1) * DQ, qsl])
                            av_ps = avp.tile([P, QT], F32, tag="av")
                            dn_ps = dnp.tile([1, QT], F32, tag="dn")
                            nkb = 4 * qt + 4
                            for kb in range(nkb):
                                ksl = slice(kb * P, (kb + 1) * P)
                                sc = scp.tile([P, QT], F32, tag="sc")
                                nc.tensor.matmul(sc[:], knope_sb[h][:, ksl],
                                                 qn[:], start=True, stop=False)
                                nc.tensor.matmul(sc[:], kpe_sb[:, ksl], qp[:],
                                                 start=False, stop=True)
                                if kb >= 4 * qt:
                                    i = kb - 4 * qt
                                    nc.vector.tensor_tensor(
                                        sc[:], sc[:],
                                        maskp_sb[:, i * QT:(i + 1) * QT],
                                        OP.add)
                                e = b5e.tile([P, QT], F32R, tag="e")
                                nc.scalar.activation(e[:], sc[:], AF.Exp)
                                nc.tensor.matmul(
                                    dn_ps[:], ones128[:], e[:],
                                    start=(kb == 0), stop=(kb == nkb - 1),
                                    skip_group_check=True)
                                nc.tensor.matmul(
                                    av_ps[:],
                                    v_sb[:, kb * HPC * DV + h * DV:
                                         kb * HPC * DV + (h + 1) * DV],
                                    e[:], start=(kb == 0),
                                    stop=(kb == nkb - 1),
                                    skip_group_check=True)
                            rden = b5t.tile([1, QT], F32R, tag="rden")
                            with nc.allow_low_precision(
                                    reason="f32r rounding of 1/den intended"):
                                nc.vector.reciprocal(rden[:], dn_ps[:])
                            bc = bcp.tile([P, QT], F32, tag="bc")
                            nc.tensor.matmul(bc[:], ones1[:], rden[:],
                                             start=True, stop=True,
                                             skip_group_check=True)
                            bc_sb = b5t.tile([P, QT], F32, tag="bcsb")
                            nc.scalar.copy(bc_sb[:], bc[:])
                            o_sb = b5t.tile([P, QT], F32R, tag="osb")
                            nc.vector.tensor_tensor(o_sb[:], av_ps[:],
                                                    bc_sb[:], OP.mult)
                            nc.sync.dma_start(
                                o_bounce[h * DV:(h + 1) * DV, qsl], o_sb[:])

            # ---------------- all-gather attention outputs ------------------
            nc.gpsimd.collective_compute(
                "AllGather", OP.bypass, replica_groups=GROUPS,
                ins=[o_bounce[:].opt()], outs=[o_gath[:].opt()])

            # ---------------- stage C: column-parallel o_proj ---------------
            with (
                tc.tile_pool(name="c_w", bufs=1) as cw,
                tc.tile_pool(name="c_r", bufs=17) as crp,
                tc.tile_pool(name="c_ps", bufs=2, space="PSUM") as cps,
                tc.tile_pool(name="c_t", bufs=3) as ctp,
            ):
                wo_sb = [cw.tile([P, KH * P], F32R, tag=f"wo{m}",
                                 name=f"wo_sb{m}")
                         for m in range(LANES)]
                for m in range(LANES):
                    nc.sync.dma_start(wo_sb[m][:], wo_t[m * P:(m + 1) * P, :])
                for n in range(NQT):
                    nsl = slice(n * QT, (n + 1) * QT)
                    rh = []
                    for k in range(KH):
                        t = crp.tile([P, QT], F32R, tag="oh")
                        nc.sync.dma_start(t[:], o_gath[k * P:(k + 1) * P, nsl])
                        rh.append(t)
                    for m in range(LANES):
                        ps = cps.tile([P, QT], F32, tag="cps")
                        for k in range(KH):
                            nc.tensor.matmul(ps[:],
                                             wo_sb[m][:, k * P:(k + 1) * P],
                                             rh[k][:], start=(k == 0),
                                             stop=(k == KH - 1))
                        ot_sb = ctp.tile([P, QT], F32, tag="ot")
                        nc.vector.tensor_copy(ot_sb[:], ps[:])
                        nc.sync.dma_start(o_t[m * P:(m + 1) * P, nsl],
                                          ot_sb[:])

    nc.compile()
    return nc


def _get_nc():
    global _NC_CACHE
    if _NC_CACHE is None:
        _NC_CACHE = _build()
    return _NC_CACHE


def _strips(wT, nk, nm):
    """[K, M] weight (already transposed, contraction-major) -> row-blocked
    lhsT strips layout [nm*P, nk*P]: row m*P+p, col k*P+j = wT[k*P+p, m*P+j]."""
    return np.ascontiguousarray(
        wT.reshape(nk, P, nm, P).transpose(2, 1, 0, 3).reshape(nm * P, nk * P))


def kernel(hidden_states, attention_mask, position_ids, wq_a, q_a_ln_w, wq_b,
           wkv_a, kv_a_ln_w, wkv_b, wo):
    hidden_states = np.asarray(hidden_states, np.float32)
    attention_mask = np.asarray(attention_mask, np.float32)
    position_ids = np.asarray(position_ids)
    wq_a = np.asarray(wq_a, np.float32)
    q_a_ln_w = np.asarray(q_a_ln_w, np.float32)
    wq_b = np.asarray(wq_b, np.float32)
    wkv_a = np.asarray(wkv_a, np.float32)
    kv_a_ln_w = np.asarray(kv_a_ln_w, np.float32)
    wkv_b = np.asarray(wkv_b, np.float32)
    wo = np.asarray(wo, np.float32)

    perm = np.array([2 * (j % 32) + j // 32 for j in range(DR)])

    # ---- shared (head-independent) weights ----
    wqa_arr = _strips(np.ascontiguousarray(wq_a.T), KH, KQ)
    wkva_main = np.ascontiguousarray(wkv_a[:KVLR].T)
    wkva_arr = _strips(wkva_main, KH, KC)
    pe_dev = wkv_a[KVLR:][perm]
    pe_rot = np.concatenate([-pe_dev[DR // 2:], pe_dev[:DR // 2]], axis=0)
    wkva_pe_arr = _strips(
        np.ascontiguousarray(np.concatenate([pe_dev, pe_rot], axis=0).T),
        KH, 1)

    # ---- per-lane (head-group) weights ----
    wq_b_eff = (wq_b * q_a_ln_w[None, :]) * (DQ ** -0.5)
    wkv_b_eff = wkv_b * kv_a_ln_w[None, :]
    lane_arrs = []
    for l in range(LANES):
        heads = range(HPC * l, HPC * (l + 1))
        qb_rows = []
        for h in heads:
            nope = wq_b_eff[DQ * h:DQ * h + DQN]
            pe = wq_b_eff[DQ * h + DQN:DQ * (h + 1)][perm]
            rot = np.concatenate([-pe[DR // 2:], pe[:DR // 2]], axis=0)
            qb_rows += [nope, pe, rot]
        wqb_arr = _strips(
            np.ascontiguousarray(np.concatenate(qb_rows, axis=0).T), KQ,
            2 * HPC)
        kn_rows = np.concatenate(
            [wkv_b_eff[(DQN + DV) * h:(DQN + DV) * h + DQN] for h in heads],
            axis=0)
        wkvbk_arr = _strips(np.ascontiguousarray(kn_rows.T), KC, HPC)
        v_rows = np.concatenate(
            [wkv_b_eff[(DQN + DV) * h + DQN:(DQN + DV) * (h + 1)]
             for h in heads], axis=0)
        wv_arr = np.ascontiguousarray(v_rows.T)                # [KVLR, 512]
        wo_arr = _strips(
            np.ascontiguousarray(wo[QT * l:QT * (l + 1)].T), KH, LANES)
        lane_arrs.append(dict(wqb_t=wqb_arr, wkvb_k=wkvbk_arr, wv=wv_arr,
                              wo_t=wo_arr))

    # ---- per-group (batch) arrays ----
    inv_freq = (1.0 / (BASE ** (np.arange(0, DR, 2, dtype=np.float32) / DR))
                ).astype(np.float32)
    group_arrs = []
    for g in range(B):
        freqs = position_ids[g].astype(np.float32)[:, None] * inv_freq[None, :]
        c = np.cos(freqs).astype(np.float32).T           # [32, S]
        s = np.sin(freqs).astype(np.float32).T
        cos_qa = np.ascontiguousarray(np.concatenate([c, c], axis=0))
        sin_qa = np.ascontiguousarray(np.concatenate([s, s], axis=0))
        m0 = attention_mask[g, 0]                         # [S, S] additive
        patt = np.ascontiguousarray(np.concatenate(
            [m0[0:QT, off:off + P].T for off in range(0, QT, P)], axis=0))
        hid_tg = np.ascontiguousarray(hidden_states[:, g, :].T)  # [HID, S]
        group_arrs.append(dict(cos_q=cos_qa, sin_q=sin_qa, maskp=patt,
                               hid=hid_tg))

    in_maps = []
    for cidx in range(N_CORES):
        g, l = divmod(cidx, LANES)
        ga = group_arrs[g]
        tsl = slice(TOKS * l, TOKS * (l + 1))
        m = dict(
            hid_t=np.ascontiguousarray(ga["hid"][:, tsl]),
            wqa_t=wqa_arr, wkva_t=wkva_arr, wkva_pe=wkva_pe_arr,
            cos_k=np.ascontiguousarray(ga["cos_q"][:, tsl]),
            sin_k=np.ascontiguousarray(ga["sin_q"][:, tsl]),
            cos_q=ga["cos_q"], sin_q=ga["sin_q"], maskp=ga["maskp"],
            **lane_arrs[l],
        )
        in_maps.append(m)

    res = run_bass_kernel_spmd(_get_nc(), in_maps,
                               core_ids=list(range(N_CORES)))

    out = np.empty((S, B, HID), np.float32)
    for cidx in range(N_CORES):
        g, l = divmod(cidx, LANES)
        out[:, g, QT * l:QT * (l + 1)] = res.results[cidx]["o_t"].T
    return out


if __name__ == "__main__":
    import reference
    inputs = {k: np.asarray(v) for k, v in reference.setup_inputs().items()}
    outp = kernel(**inputs)
    print("kernel output", outp.shape, outp.dtype)
